# revision 28
# baseline (speedup 1.0000x reference)
"""MLA attention Trainium2 kernel: nn_MultiHeadLatentAttention_31722628448847.

Full computation (B=1, T=2048, C=2048, H=16, G=4, Dl=32):
  q  = x @ Wq.T   -> [T, H, G, Dl]
  lk = x @ Wlk.T  -> [T, H, Dl]
  lv = x @ Wlv.T  -> [T, H, Dl]
  scores[h,g,t,s] = (q[t,h,g,:] . lk[s,h,:]) / sqrt(128)
  probs = softmax_s(scores)
  attn[t, h,g,:] = sum_s probs * lv[s,h,:]
  out = attn @ Wo.T

Sharding: 2 heads per core (8 cores); each core computes a full-width
partial of the output projection in bf16; partials are summed on the host
in f32.

v2 design notes (vs the 511us baseline):
  - scores: all 4 group matmuls (K=32) issued back-to-back at row
    tile_positions (32g, 0) so they execute concurrently in the PE array.
  - exp: split between ScalarE (table exp) and DVE (Schraudolph-style
    PWL exp: int16 = round(A*score + B) bit-viewed as bf16), tunable via
    DVE_EXP_8THS. Both engines run in parallel with the PE.
  - kv projections packed into one [*, 128] matmul (wk+wv columns).
  - AV: lv-stationary with the ones-column denominator trick; h0/h1 at
    column tile_positions (0,0)/(0,64) for pairwise concurrency.
  - softmax normalization: reciprocal straight off PSUM (DVE), f32r
    ones-broadcast matmul for the per-(g,t) scale, DVE multiply.
  - output projection partials written as bf16 (halves DMA + host sum).
  - attention t-chunk TA=256 keeps PSUM inside 8 banks with sc bufs=3.
"""

import numpy as np

T = 2048
C = 2048
HPC = 2  # heads per core
DH = 128
DL = 32
G = 4
N_CORES = 8
TP = 512  # projection free-dim chunk
TA = 512  # attention t-chunk
SCALE = 1.0 / np.sqrt(np.float32(DH))

# PWL exp constants: int16 = round(score * A + B), bits viewed as bf16
A_PWL = float(SCALE * 128.0 / np.log(2.0))
B_PWL = float(127 * 128 - 7.5)
# exp tiles with (sb*2+h) % 8 < DVE_EXP_8THS go to DVE-PWL, rest to ScalarE
import os

DVE_EXP_8THS = int(os.environ.get("DVE_EXP_8THS", "4"))
TRUNC_TA = int(os.environ.get("TRUNC_TA", "0"))  # 0 = all chunks
TRUNC_STAGE = int(os.environ.get("TRUNC_STAGE", "4"))  # 1=scores 2=+av 3=+norm 4=full
SC_SPREAD = int(os.environ.get("SC_SPREAD", "0"))  # 1: one score MM per PSUM bank
SCP_BUFS = int(os.environ.get("SCP_BUFS", "2"))
NORM_MODE = os.environ.get("NORM_MODE", "gpsimd")  # gpsimd | pe_f32


def build_program(t=T, c=C):
    import concourse.mybir as mybir
    import concourse.tile as tile
    from concourse import bacc
    from concourse.masks import make_identity

    bf16 = mybir.dt.bfloat16
    f32 = mybir.dt.float32
    f32r = mybir.dt.float32r
    i16 = mybir.dt.int16
    EXP = mybir.ActivationFunctionType.Exp

    n_cb = c // 128  # contraction blocks
    n_sb = t // 128  # s-blocks
    n_ta = t // TA  # attention t-chunks
    QCOLS = HPC * DH  # 256
    KVCOLS = 2 * HPC * DL  # 128 (wk 0:64, wv 64:128)

    nc = bacc.Bacc("TRN2", target_bir_lowering=False, debug=False, num_devices=1)

    xT_d = nc.dram_tensor("xT", [c, t], bf16, kind="ExternalInput").ap()
    wqT_d = nc.dram_tensor("wqT", [c, QCOLS], bf16, kind="ExternalInput").ap()
    wkvT_d = nc.dram_tensor("wkvT", [c, KVCOLS], bf16, kind="ExternalInput").ap()
    woT_d = nc.dram_tensor("woT", [QCOLS, c], bf16, kind="ExternalInput").ap()
    out_d = nc.dram_tensor("out", [t, c], bf16, kind="ExternalOutput").ap()
    taps = None
    if os.environ.get("DEBUG_TAPS"):
        taps = {
            "qT0": nc.dram_tensor("qT0", [128, t], bf16, kind="ExternalOutput").ap(),
            "lkT0": nc.dram_tensor("lkT0", [128, t], bf16, kind="ExternalOutput").ap(),
            "lv0": nc.dram_tensor("lv0", [128, 66], bf16, kind="ExternalOutput").ap(),
            "exp0": nc.dram_tensor("exp0", [128, G * TA], bf16, kind="ExternalOutput").ap(),
            "at0": nc.dram_tensor("at0", [128, TA], bf16, kind="ExternalOutput").ap(),
            "bc0": nc.dram_tensor("bc0", [DL, 2 * TA], mybir.dt.float32, kind="ExternalOutput").ap(),
        }

    with tile.TileContext(nc) as tc_:
        _emit(
            nc, tc_, tile, mybir, make_identity, bf16, f32, f32r, i16, EXP,
            xT_d, wqT_d, wkvT_d, woT_d, out_d,
            t, c, n_cb, n_sb, n_ta, QCOLS, KVCOLS, taps,
        )
    nc.compile()
    return nc


def _emit_av(nc, av, lv_sb, pend):
    exp_t, h, sb, gp = pend
    last = sb == T // 128 - 1 and h == HPC - 1
    for gi in range(2):
        g = gp * 2 + gi
        nc.tensor.matmul(
            av[g][h * 64 : h * 64 + DL + 1, :],
            lv_sb[sb][:, h * (DL + 1) : (h + 1) * (DL + 1)],
            exp_t[:, gi * 512 : (gi + 1) * 512],
            start=False,
            stop=last,
            skip_group_check=True,
            tile_position=(0, h * 64),
        )


def _emit(
    nc, tc_, tile, mybir, make_identity, bf16, f32, f32r, i16, EXP,
    xT_d, wqT_d, wkvT_d, woT_d, out_d,
    t, c, n_cb, n_sb, n_ta, QCOLS, KVCOLS, taps=None,
):
    from contextlib import ExitStack

    H = HPC
    n_tp = t // TP
    MUL = mybir.AluOpType.mult
    ADD = mybir.AluOpType.add

    ctx = ExitStack()
    with ctx:
        # ---------------- persistent SBUF inputs ----------------
        wpool = ctx.enter_context(tc_.tile_pool(name="wpool", bufs=1))
        xT_sb = []
        wqT_sb = []
        wkvT_sb = []
        for kb in range(n_cb):
            xt = wpool.tile([128, t], bf16, name=f"xT{kb}")
            nc.sync.dma_start(xt[:], xT_d[kb * 128 : (kb + 1) * 128, :])
            xT_sb.append(xt)
            wq = wpool.tile([128, QCOLS], bf16, name=f"wqT{kb}")
            nc.sync.dma_start(wq[:], wqT_d[kb * 128 : (kb + 1) * 128, :])
            wqT_sb.append(wq)
            wkv = wpool.tile([128, KVCOLS], bf16, name=f"wkvT{kb}")
            nc.sync.dma_start(wkv[:], wkvT_d[kb * 128 : (kb + 1) * 128, :])
            wkvT_sb.append(wkv)
        woT_sb = []
        for h in range(H):
            wo = wpool.tile([128, c], bf16, name=f"woT{h}")
            nc.sync.dma_start(wo[:], woT_d[h * 128 : (h + 1) * 128, :])
            woT_sb.append(wo)

        ident = wpool.tile([128, 128], bf16, name="ident")
        make_identity(nc, ident[:])
        ones_f = wpool.tile([1, DL], f32, name="ones_f")
        nc.any.memset(ones_f[:], 1.0)

        # ---------------- projection outputs (SBUF) ----------------
        apool = ctx.enter_context(tc_.tile_pool(name="apool", bufs=1))
        qT = [apool.tile([128, t], bf16, name=f"qT{h}") for h in range(H)]
        lkpad = [
            [apool.tile([128, t], bf16, name=f"lkp{h}{g}") for g in range(G)]
            for h in range(H)
        ]
        for h in range(H):
            for g in range(G):
                nc.gpsimd.memset(lkpad[h][g][:], 0.0)
        # lv per s-block: [128 s, 66]: cols 0-31 lv_h0, col 32 ones,
        # cols 33-64 lv_h1, col 65 ones
        lv_all = apool.tile([128, 66 * n_sb], bf16, name="lv_all")
        lv_sb = [lv_all[:, 66 * sb : 66 * (sb + 1)] for sb in range(n_sb)]
        lvT_tmp = apool.tile([2 * HPC * DL // 2, t], bf16, name="lvT_tmp")  # [64, t]

        # ---------------- projections ----------------
        pctx = ExitStack()
        ppool = pctx.enter_context(tc_.tile_pool(name="ppool", bufs=2, space="PSUM"))
        # kv first (attention needs all of lk/lv)
        pss_kv = [
            ppool.tile([KVCOLS, TP], f32, name=f"ps_kv{n}", tag=f"pp{n}", bufs=1)
            for n in range(n_tp)
        ]
        for kb in range(n_cb):
            for nch in range(n_tp):
                nc.tensor.matmul(
                    pss_kv[nch][:],
                    wkvT_sb[kb][:],
                    xT_sb[kb][:, nch * TP : (nch + 1) * TP],
                    start=(kb == 0),
                    stop=(kb == n_cb - 1),
                )
        # lk: psum rows h*32..h*32+32 -> 4 replicated g-strips per head
        for nch in range(n_tp):
            for h in range(H):
                for g in range(G):
                    nc.vector.tensor_copy(
                        lkpad[h][g][g * DL : (g + 1) * DL, nch * TP : (nch + 1) * TP],
                        pss_kv[nch][h * DL : (h + 1) * DL, :],
                    )
            # lv rows 64:128 -> transposed staging
            nc.vector.tensor_copy(
                lvT_tmp[:, nch * TP : (nch + 1) * TP],
                pss_kv[nch][2 * DL : 4 * DL, :],
            )
        # lv natural layout via PE transpose
        for sb in range(n_sb):
            pt = ppool.tile([128, 2 * DL], bf16, name="ps_t", tag="pt")
            nc.tensor.transpose(
                pt[:], lvT_tmp[:, sb * 128 : (sb + 1) * 128], ident[0 : 2 * DL, 0 : 2 * DL]
            )
            nc.vector.tensor_copy(lv_sb[sb][:, 0:DL], pt[:, 0:DL])
            nc.vector.tensor_copy(lv_sb[sb][:, DL + 1 : 2 * DL + 1], pt[:, DL : 2 * DL])
        for sb in range(n_sb):
            nc.vector.memset(lv_sb[sb][:, DL : DL + 1], 1.0)
            nc.vector.memset(lv_sb[sb][:, 2 * DL + 1 : 2 * DL + 2], 1.0)
        # q projection: chunk 0 only; later chunks are emitted at the
        # preceding attention chunk boundary as PE filler during norm.
        ps_q0 = ppool.tile([128, 1024], f32, name="ps_q0", tag="pp0", bufs=1)
        for h in range(H):
            for kb in range(n_cb):
                nc.tensor.matmul(
                    ps_q0[:, h * TA : (h + 1) * TA],
                    wqT_sb[kb][:, h * 128 : (h + 1) * 128],
                    xT_sb[kb][:, 0:TA],
                    start=(kb == 0),
                    stop=(kb == n_cb - 1),
                )
        for h in range(H):
            nc.vector.tensor_copy(qT[h][:, 0:TA], ps_q0[:, h * TA : (h + 1) * TA])
        pctx.close()
        if taps is not None:
            nc.sync.dma_start(taps["qT0"][:, :], qT[0][:])
            nc.sync.dma_start(taps["lkT0"][:, :], lkT[0][:])
            nc.sync.dma_start(taps["lv0"][:, :], lv_all[:, 0:66])

        # ---------------- attention + output projection ----------------
        scp = ctx.enter_context(tc_.tile_pool(name="scp", bufs=SCP_BUFS, space="PSUM"))
        avp = ctx.enter_context(tc_.tile_pool(name="avp", bufs=1, space="PSUM"))
        expool = ctx.enter_context(tc_.tile_pool(name="expool", bufs=6))
        atpool = ctx.enter_context(tc_.tile_pool(name="atpool", bufs=2 * H))
        rpool = ctx.enter_context(tc_.tile_pool(name="rpool", bufs=1))
        opool = ctx.enter_context(tc_.tile_pool(name="opool", bufs=4))

        for ta in range(TRUNC_TA if TRUNC_TA else n_ta):
            tsl = slice(ta * TA, (ta + 1) * TA)
            # AV accumulators: av[gp] [128, 512]: cols gi*256+t;
            # rows h*64+0:32 = attnU, row h*64+32 = denom
            av = [
                avp.tile([128, TA], f32, name=f"av{g}", tag=f"av{g}")
                for g in range(G)
            ]
            for g in range(G):
                nc.vector.memset(av[g][:], 0.0)
            # pipeline units: (sb, h, gpair). AV emission delayed one unit so
            # scores(n+1) never queues behind AV(n) waiting on exp(n).
            pend = None  # (exp_t, h, sb, gp)
            uidx = 0
            for sb in range(n_sb):
                for h in range(H):
                    for gp in range(2):
                        sc = scp.tile([128, 1024], f32, name="sc", tag="sc")
                        for gi in range(2):
                            g = gp * 2 + gi
                            nc.tensor.matmul(
                                sc[:, gi * 512 : (gi + 1) * 512],
                                lkpad[h][g][:, sb * 128 : (sb + 1) * 128],
                                qT[h][:, tsl],
                                start=True,
                                stop=True,
                            )
                        exp_t = expool.tile([128, 1024], bf16, name="exp_t", tag="ex")
                        if ((uidx * DVE_EXP_8THS) % 8) < DVE_EXP_8THS and DVE_EXP_8THS:
                            nc.vector.tensor_scalar(
                                exp_t[:].bitcast(i16), sc[:], A_PWL, B_PWL, MUL, ADD
                            )
                        else:
                            nc.scalar.activation(exp_t[:], sc[:], EXP, scale=float(SCALE))
                        uidx += 1
                        if pend is not None and TRUNC_STAGE >= 2:
                            _emit_av(nc, av, lv_sb, pend)
                        pend = (exp_t, h, sb, gp)
            if pend is not None and TRUNC_STAGE >= 2:
                _emit_av(nc, av, lv_sb, pend)
            # q projection for the next chunk: independent PE work that
            # fills the norm/outproj dependency gap and keeps HAM warm.
            if ta + 1 < n_ta:
                nsl = slice((ta + 1) * TA, (ta + 2) * TA)
                ps_q = scp.tile([128, 1024], f32, name="ps_q", tag="sc")
                for h in range(H):
                    for kb in range(n_cb):
                        nc.tensor.matmul(
                            ps_q[:, h * TA : (h + 1) * TA],
                            wqT_sb[kb][:, h * 128 : (h + 1) * 128],
                            xT_sb[kb][:, nsl],
                            start=(kb == 0),
                            stop=(kb == n_cb - 1),
                        )
                for h in range(H):
                    nc.vector.tensor_copy(qT[h][:, nsl], ps_q[:, h * TA : (h + 1) * TA])
            # normalization -> attnT bf16 [(g,d), t] per head
            at = [atpool.tile([128, TA], bf16, name=f"at{h}", tag=f"at{h}") for h in range(H)]
            if TRUNC_STAGE < 3:
                continue
            bcs = {}
            for h in range(H):
                for gp in range(2):
                    den = rpool.tile([1, 2 * TA], f32, name=f"den{h}{gp}", tag=f"den{h}{gp}")
                    nc.scalar.copy(den[:, 0:TA], av[2 * gp][h * 64 + DL : h * 64 + DL + 1, :])
                    nc.scalar.copy(den[:, TA : 2 * TA], av[2 * gp + 1][h * 64 + DL : h * 64 + DL + 1, :])
                    rec = rpool.tile([1, 2 * TA], f32, name=f"rec{h}{gp}", tag=f"rec{h}{gp}")
                    nc.vector.reciprocal_approx_fast(rec[:], den[:])
                    bcb = rpool.tile([DL, 2 * TA], f32, name=f"bc{h}{gp}", tag=f"bc{h}{gp}")
                    bcs[(h, gp)] = bcb
                    if NORM_MODE == "gpsimd":
                        nc.gpsimd.partition_broadcast(bcb[:], rec[:])
                    else:
                        bcp = scp.tile([128, 1024], f32, name="bcp", tag="sc")
                        nc.tensor.matmul(
                            bcp[0:DL, 0 : 2 * TA], ones_f[:], rec[:],
                            start=True, stop=True,
                        )
                        nc.vector.tensor_copy(bcb[:], bcp[0:DL, 0 : 2 * TA])
            for h in range(H):
                for g in range(G):
                    gp, gi = g // 2, g % 2
                    nc.vector.tensor_tensor(
                        at[h][g * DL : (g + 1) * DL, :],
                        av[g][h * 64 : h * 64 + DL, :],
                        bcs[(h, gp)][:, gi * TA : (gi + 1) * TA],
                        MUL,
                    )
            if taps is not None and ta == 0:
                nc.sync.dma_start(taps["at0"][:, :], at[0][:])
                nc.sync.dma_start(taps["bc0"][:, :], bcs[(0, 0)][:])
            # output projection for this t-chunk
            for tb in range(TA // 128 if TRUNC_STAGE >= 4 else 0):
                t0 = ta * TA + tb * 128
                wop = [scp.tile([128, 1024], f32, name=f"wop{oc}", tag="sc") for oc in range(2)]
                for h in range(H):
                    for oc in range(2):
                        for nh in range(2):
                            nc.tensor.matmul(
                                wop[oc][:, nh * 512 : (nh + 1) * 512],
                                at[h][:, tb * 128 : (tb + 1) * 128],
                                woT_sb[h][:, oc * 1024 + nh * 512 : oc * 1024 + (nh + 1) * 512],
                                start=(h == 0),
                                stop=(h == H - 1),
                            )
                for oc in range(2):
                    ob = opool.tile([128, 1024], bf16, name="ob", tag="ob")
                    if oc == 0:
                        nc.scalar.copy(ob[:], wop[oc][:])
                    else:
                        nc.vector.tensor_copy(ob[:], wop[oc][:])
                    nc.sync.dma_start(
                        out_d[t0 : t0 + 128, oc * 1024 : (oc + 1) * 1024], ob[:]
                    )


# ---------------- host side ----------------


def shard_inputs(x, Wq, Wlk, Wlv, Wo):
    """Returns per-core input dicts (bf16, pre-transposed)."""
    import ml_dtypes

    bf = ml_dtypes.bfloat16
    X = np.ascontiguousarray(x.reshape(-1, x.shape[-1]))  # [T, C]
    xT = np.ascontiguousarray(X.T).astype(bf)
    maps = []
    for core in range(N_CORES):
        h0 = core * HPC
        qr = slice(h0 * DH, (h0 + HPC) * DH)
        kr = slice(h0 * DL, (h0 + HPC) * DL)
        wkv = np.concatenate([Wlk[kr, :], Wlv[kr, :]], axis=0)  # [128, C]
        maps.append(
            {
                "xT": xT,
                "wqT": np.ascontiguousarray(Wq[qr, :].T).astype(bf),
                "wkvT": np.ascontiguousarray(wkv.T).astype(bf),
                "woT": np.ascontiguousarray(Wo[:, qr].T).astype(bf),
            }
        )
    return maps


_CACHE = {}


def kernel(x, Wq, Wk, Wv, Wlk, Wlv, Wo):
    """Full-input entry point. Wk/Wv are unused by the reference forward."""
    if "nc" not in _CACHE:
        _CACHE["nc"] = build_program()
    nc = _CACHE["nc"]
    from concourse.bass_utils import run_bass_kernel_spmd

    in_maps = shard_inputs(
        np.asarray(x, dtype=np.float32),
        np.asarray(Wq, dtype=np.float32),
        np.asarray(Wlk, dtype=np.float32),
        np.asarray(Wlv, dtype=np.float32),
        np.asarray(Wo, dtype=np.float32),
    )
    res = run_bass_kernel_spmd(nc, in_maps, list(range(N_CORES)))
    out = np.zeros((T, C), dtype=np.float32)
    for r in res.results:
        out += np.asarray(r["out"], dtype=np.float32)
    return out.reshape(1, T, C)


def _cache_get():
    return _CACHE["nc"]
